# revision 3
# baseline (speedup 1.0000x reference)
"""Trainium2 Bass kernel for single-head attention (B=8, N=2048, C=512).

Strategy: data-parallel over batch across the 8 NeuronCores — each core
computes one full batch sample. The whole chain is laid out so that NO
on-device transposes are needed:

  per core (b = core id):
    qT[d,n] = (SCALE*w_q) @ x_b^T        (lhsT = w_qT tiles,  rhs = xT)
    kT[d,n] = w_k @ x_b^T                (lhsT = w_kT tiles,  rhs = xT)
    v[m,d]  = x_b @ w_v^T                (lhsT = xT tiles,    rhs = w_vT)
    ST[m,n] = kT^T-tiles @ qT            (= scores transposed, no max-sub)
    PT[m,n] = exp(ST)                    (ACT, PSUM -> SBUF fp32r)
    avT[d,n] = sum_m v-tile^T @ PT       (= (P@V)^T, unnormalized)
    s[n]    = sum_m ones^T @ PT          (softmax denominators, M=1 matmul)
    yT[e,n] = w_p @ avT  * (1/s[n])      (recip broadcast via ones matmul)
  host: out[b] = yT^T + v + b_proj

All matmuls use float32r (TF32-like, 1 cycle/row at N=512; measured
~2e-4 rel err) except the projection which runs bf16 to save SBUF.
"""

import numpy as np

import concourse.bass as bass
import concourse.mybir as mybir
import concourse.tile as tile
from concourse import bacc
from concourse.bass_utils import run_bass_kernel_spmd

P = 128           # partitions
N = 2048          # tokens per batch sample
C = 512           # model dim
NT = N // P       # 16 token (m) tiles
CT = C // P       # 4 dim tiles
FB = 512          # free-dim block (n-chunk)
NCH = N // FB     # 4 n-chunks
B = 8             # batch == number of cores
SCALE = C ** -0.5

F32 = mybir.dt.float32
F32R = mybir.dt.float32r
BF16 = mybir.dt.bfloat16
EXP = mybir.ActivationFunctionType.Exp


def build():
    nc = bacc.Bacc("TRN2", target_bir_lowering=False, debug=False)

    xT = nc.dram_tensor("xT", [C, N], F32R, kind="ExternalInput")      # x[b].T
    wqT = nc.dram_tensor("wqT", [C, C], F32R, kind="ExternalInput")    # (SCALE*w_q).T [c,d]
    wkT = nc.dram_tensor("wkT", [C, C], F32R, kind="ExternalInput")    # w_k.T [c,d]
    wvT = nc.dram_tensor("wvT", [C, C], F32R, kind="ExternalInput")    # w_v.T [c,d]
    wpT = nc.dram_tensor("wpT", [C, C], F32, kind="ExternalInput")     # w_proj.T [d,e]
    yT = nc.dram_tensor("yT", [C, N], F32, kind="ExternalOutput")      # (P@V/s @ wp.T).T
    vout = nc.dram_tensor("vout", [N, C], F32, kind="ExternalOutput")  # v (for host residual)

    with tile.TileContext(nc) as tc:
        with (
            tc.tile_pool(name="sb", bufs=2) as sb,
            tc.tile_pool(name="ps", bufs=2, space="PSUM") as psp,
        ):
            # ---- weight + input loads ----
            def load_w(handle, tag, bufs, dtype=F32R):
                ws = []
                for ci in range(CT):
                    t = sb.tile([P, C], dtype, tag=tag, bufs=bufs)
                    nc.sync.dma_start(t, handle[ci * P:(ci + 1) * P, :])
                    ws.append(t)
                return ws

            # wq/wk/wv share the "blk" tag with the exp(P^T) tiles: 12 slots
            # during QKV, all 16 recycled for P tiles afterwards.
            wq = load_w(wqT, "blk", 16)
            wk = load_w(wkT, "blk", 16)
            wv = load_w(wvT, "blk", 16)
            wpf = load_w(wpT, "wpf", 2, dtype=F32)
            wpb = []
            for ci in range(CT):
                t = sb.tile([P, C], BF16, tag="wpb", bufs=4)
                nc.vector.tensor_copy(t, wpf[ci])
                wpb.append(t)

            xts = []
            for ci in range(CT):
                t = sb.tile([P, N], F32R, tag="xt", bufs=4)
                nc.sync.dma_start(t, xT[ci * P:(ci + 1) * P, :])
                xts.append(t)

            ones_f32 = sb.tile([P, 1], F32, tag="ones_f32", bufs=1)
            nc.vector.memset(ones_f32, 1.0)
            ones_col = sb.tile([P, 1], F32R, tag="ones", bufs=1)
            nc.vector.tensor_copy(ones_col, ones_f32)
            ones_row = sb.tile([1, P], F32, tag="onesr", bufs=1)
            nc.vector.memset(ones_row, 1.0)

            # ---- QKV projections ----
            qts, kts = {}, {}
            for wt, store in ((wq, qts), (wk, kts)):
                for di in range(CT):
                    for ch in range(NCH):
                        ps = psp.tile([P, FB], F32, tag="pgen", bufs=3)
                        for ci in range(CT):
                            nc.tensor.matmul(
                                ps,
                                wt[ci][:, di * P:(di + 1) * P],
                                xts[ci][:, ch * FB:(ch + 1) * FB],
                                start=(ci == 0), stop=(ci == CT - 1),
                            )
                        t = sb.tile([P, FB], F32R, tag="qk", bufs=32)
                        nc.vector.tensor_copy(t, ps)
                        store[(di, ch)] = t

            vs = []
            for mi in range(NT):
                ps = psp.tile([P, C], F32, tag="pgen", bufs=3)
                for ci in range(CT):
                    nc.tensor.matmul(
                        ps,
                        xts[ci][:, mi * P:(mi + 1) * P],
                        wv[ci],
                        start=(ci == 0), stop=(ci == CT - 1),
                    )
                t = sb.tile([P, C], F32R, tag="v", bufs=16)
                nc.vector.tensor_copy(t, ps)
                nc.sync.dma_start(vout[mi * P:(mi + 1) * P, :], t.bitcast(F32))
                vs.append(t)

            # ---- attention + projection, per n-chunk of 512 queries ----
            for ch in range(NCH):
                pavs = [
                    psp.tile([P, FB], F32, tag="pav", bufs=4, name=f"pav{ch}_{di}")
                    for di in range(CT)
                ]
                ps_s = psp.tile([1, FB], F32, tag="psum_s", bufs=1)
                for mi in range(NT):
                    psc = psp.tile([P, FB], F32, tag="pgen", bufs=3)
                    for di in range(CT):
                        nc.tensor.matmul(
                            psc,
                            kts[(di, mi // 4)][:, (mi % 4) * P:(mi % 4 + 1) * P],
                            qts[(di, ch)],
                            start=(di == 0), stop=(di == CT - 1),
                        )
                    pt = sb.tile([P, FB], F32R, tag="blk", bufs=16)
                    nc.scalar.activation(pt, psc, EXP)
                    for di in range(CT):
                        nc.tensor.matmul(
                            pavs[di],
                            vs[mi][:, di * P:(di + 1) * P],
                            pt,
                            start=(mi == 0), stop=(mi == NT - 1),
                        )
                    nc.tensor.matmul(
                        ps_s, ones_col, pt,
                        start=(mi == 0), stop=(mi == NT - 1),
                    )

                avts = []
                for di in range(CT):
                    t = sb.tile([P, FB], BF16, tag="avt", bufs=16)
                    nc.vector.tensor_copy(t, pavs[di])
                    avts.append(t)

                s_sb = sb.tile([1, FB], F32, tag="s", bufs=4)
                nc.vector.tensor_copy(s_sb, ps_s)
                # broadcast s across partitions (ones[1,128]^T @ s[1,512]),
                # then reciprocal on the full tile
                pb = psp.tile([P, FB], F32, tag="pgen", bufs=3)
                nc.tensor.matmul(pb, ones_row, s_sb, start=True, stop=True)
                rc = sb.tile([P, FB], F32, tag="rc", bufs=2)
                nc.vector.reciprocal(rc, pb)

                for ei in range(CT):
                    py = psp.tile([P, FB], F32, tag="pgen", bufs=3)
                    for di in range(CT):
                        nc.tensor.matmul(
                            py,
                            wpb[di][:, ei * P:(ei + 1) * P],
                            avts[di],
                            start=(di == 0), stop=(di == CT - 1),
                        )
                    yt = sb.tile([P, FB], F32, tag="yo", bufs=3)
                    nc.vector.tensor_mul(yt, py, rc)
                    nc.sync.dma_start(
                        yT[ei * P:(ei + 1) * P, ch * FB:(ch + 1) * FB], yt
                    )

    nc.compile()
    return nc


_NC = None


def _get_nc():
    global _NC
    if _NC is None:
        _NC = build()
    return _NC


def kernel(x, w_qkv, w_proj, b_proj):
    x = np.asarray(x, dtype=np.float32)
    w_qkv = np.asarray(w_qkv, dtype=np.float32)
    w_proj = np.asarray(w_proj, dtype=np.float32)
    b_proj = np.asarray(b_proj, dtype=np.float32)

    wq = np.ascontiguousarray((w_qkv[0:C] * SCALE).T)
    wk = np.ascontiguousarray(w_qkv[C:2 * C].T)
    wv = np.ascontiguousarray(w_qkv[2 * C:3 * C].T)
    wp = np.ascontiguousarray(w_proj.T)

    in_maps = []
    for b in range(B):
        in_maps.append({
            "xT": np.ascontiguousarray(x[b].T),
            "wqT": wq, "wkT": wk, "wvT": wv, "wpT": wp,
        })

    nc = _get_nc()
    res = run_bass_kernel_spmd(nc, in_maps, core_ids=list(range(B)))

    out = np.empty((B, N, C), np.float32)
    for b in range(B):
        out[b] = res.results[b]["yT"].T + res.results[b]["vout"] + b_proj[None, :]
    return out
